# revision 18
# baseline (speedup 1.0000x reference)
"""Deformable-conv fusion (nn_AdaptionFusion) on 8 TRN2 NeuronCores.

Data-parallel: core i handles (batch i//4, 64-col W-block i%4); weights
replicated; no collectives. See bottom for kernel() entry point.

Device pipeline per core:
  offset conv (PE, 2 psum half-passes) -> index math (DVE) ->
  idx wrap (64 fold-DMAs) -> 36x dma_gather of 2x2-pixel patch rows
  (bf16, 512B) -> bilinear lerp (DVE scalar_tensor_tensor, per-partition
  fracs) -> PE transpose to (ck, pix) -> staged to DRAM ->
  K=2304 matmul in 18 chunks (PE) -> ReLU (ACT) -> out.
"""

import sys
sys.path.insert(0, "/opt/trn_rl_repo")
import numpy as np
import ml_dtypes

import concourse.bass as bass
from concourse import bacc, mybir
from concourse.library_config import mlp

B, H, W = 2, 64, 256
C, DG, CG, K2, C_OUT = 256, 4, 64, 9, 256
WB, PAD = 64, 4
HS, WS = H + 2 * PAD, WB + 2 * PAD
TW = WS + 1
NPIX = H * WB
NROWS = HS * TW
T = 18
NBLK = 32
NGB = 3
bf16 = ml_dtypes.bfloat16

_GRAPH_CACHE = {}


def build_graph():
    f32, i16, bf = mybir.dt.float32, mybir.dt.int16, mybir.dt.bfloat16
    nc = bacc.Bacc("TRN2", target_bir_lowering=False, num_swdge_queues=4)

    x_off = nc.declare_dram_parameter("x_off", [128, 2, NPIX], f32, isOutput=False)
    ptab = nc.declare_dram_parameter("ptab", [DG, NROWS, 256], bf, isOutput=False)
    w_off_p = nc.declare_dram_parameter("w_off_p", [128, 2, 72], f32, isOutput=False)
    gridbias = nc.declare_dram_parameter("gridbias", [128, 32, 72], f32, isOutput=False)
    w_def_p = nc.declare_dram_parameter("w_def_p", [128, 18, 256], bf, isOutput=False)
    ident_e = nc.declare_dram_parameter("ident", [128, 128], bf, isOutput=False)
    out_ext = nc.declare_dram_parameter("out", [128, 2, NPIX], f32, isOutput=True)
    st_dram = nc.dram_tensor("st_dram", [T, 128, NPIX], bf)

    xo_sb = nc.alloc_sbuf_tensor("xo_sb", [128, 2, NPIX], f32)
    wo_sb = nc.alloc_sbuf_tensor("wo_sb", [128, 2, 72], f32)
    gb_sb = nc.alloc_sbuf_tensor("gb_sb", [128, 32, 72], f32)
    wd_sb = nc.alloc_sbuf_tensor("wd_sb", [128, 18, 256], bf)
    id_sb = nc.alloc_sbuf_tensor("id_sb", [128, 128], bf)
    arena2 = nc.alloc_sbuf_tensor("arena2", [128, 5760], f32)
    arena2_i = arena2.bitcast(i16)          # pitch 11520 i16
    xo_i = xo_sb.bitcast(i16)               # pitch 16384 i16
    wf_sb = nc.alloc_sbuf_tensor("wf_sb", [128, 32, 72], bf)
    li_sb = nc.alloc_sbuf_tensor("li_sb", [128, 32, 36], f32)
    li16_sb = nc.alloc_sbuf_tensor("li16_sb", [128, 32, 36], i16)

    def _v(tensor, pitch, off, dims):
        return bass.AP(tensor=tensor, offset=off, ap=[[pitch, 128]] + dims)

    # overlays: p/fi/ff share arena2 with idxw (idxw written after all dead)
    p_ap = _v(arena2, 5760, 0, [[72, 32], [1, 72]])
    fi_ap = _v(arena2_i, 11520, 4608, [[72, 32], [1, 72]])
    ff_ap = _v(arena2, 5760, 3456, [[72, 32], [1, 72]])
    idxw_ap = _v(arena2_i, 11520, 0, [[256, 36], [1, 256]])
    # idxI overlays xo (xo dead after offset conv)
    idxI_ap = _v(xo_i, 16384, 0, [[36 * 32, 8], [32, 36], [1, 32]])
    tmp_sb = nc.alloc_sbuf_tensor("tmp_sb", [128, 128], bf)
    gbuf = [nc.alloc_sbuf_tensor(f"gbuf{i}", [128, 32, 256], bf) for i in range(NGB)]
    samp = [nc.alloc_sbuf_tensor(f"samp{i}", [128, 32, 128], bf) for i in range(2)]
    st_sb = [nc.alloc_sbuf_tensor(f"st{i}", [128, NPIX], bf) for i in range(2)]
    rb_sb = [nc.alloc_sbuf_tensor(f"rb{i}", [128, 1024], bf) for i in range(2)]
    ob_sb = nc.alloc_sbuf_tensor("ob_sb", [128, 2, 1024], f32)

    ps_sh = nc.alloc_psum_tensor("ps_sh", [128, 16, 128], f32)   # 8KB banks 0-3
    ps_tr = [nc.alloc_psum_tensor(f"ps_tr{i}", [128, 1024], bf) for i in range(2)]
    ps_out = ps_sh.ap().rearrange("p a b -> p (a b)").rearrange(
        "p (r n) -> p r n", r=4)                                  # [128, 4, 512] view

    with (
        nc.Block() as block,
        nc.semaphore("in_sem") as in_sem,
        nc.semaphore("off_sem") as off_sem,
        nc.semaphore("pm_sem") as pm_sem,
        nc.semaphore("idx_sem") as idx_sem,
        nc.semaphore("wrap_sem") as wrap_sem,
        nc.semaphore("g_sem0") as g_sem0,
        nc.semaphore("g_sem1") as g_sem1,
        nc.semaphore("g_sem2") as g_sem2,
        nc.semaphore("g_sem3") as g_sem3,
        nc.semaphore("lerp_sem") as lerp_sem,
        nc.semaphore("tr_sem") as tr_sem,
        nc.semaphore("ev_sem") as ev_sem,
        nc.semaphore("stw_sem") as stw_sem,
        nc.semaphore("rb_sem0") as rb_sem0,
        nc.semaphore("rb_sem1") as rb_sem1,
        nc.semaphore("mm_sem") as mm_sem,
        nc.semaphore("prep_sem") as prep_sem,
        nc.semaphore("vs_sem") as vs_sem,
        nc.semaphore("ob_sem") as ob_sem,
        nc.semaphore("out_sem") as out_sem,
    ):
        g_sems = [g_sem0, g_sem1, g_sem2, g_sem3]
        rb_sems = [rb_sem0, rb_sem1]

        @block.sync
        def _(sync):
            sync.dma_start(out=xo_sb.ap(), in_=x_off[:]).then_inc(in_sem, 16)
            sync.dma_start(out=wo_sb.ap(), in_=w_off_p[:]).then_inc(in_sem, 16)
            sync.dma_start(out=gb_sb.ap(), in_=gridbias[:]).then_inc(in_sem, 16)
            sync.dma_start(out=wd_sb.ap(), in_=w_def_p[:]).then_inc(in_sem, 16)
            sync.dma_start(out=id_sb.ap(), in_=ident_e[:]).then_inc(in_sem, 16)
            # idx wrap: idxw[16c+q, t, 8i+u] = li16[16u+q, i, t]
            sync.wait_ge(idx_sem, 1)
            # hop 1: idxI[16c+q, u, t, i] = li16[16u+q, i, t]
            for c in range(8):
                for u in range(8):
                    src = bass.AP(
                        tensor=li16_sb, offset=(16 * u) * (32 * 36),
                        ap=[[32 * 36, 16], [36, 32], [1, 36]])
                    dst = bass.AP(
                        tensor=xo_i, offset=(16 * c) * 16384 + u * (36 * 32),
                        ap=[[16384, 16], [36, 32], [1, 36]])
                    sync.dma_start(out=dst, in_=src).then_inc(wrap_sem, 16)
            # st writes (HWDGE): st[t] once its evictions are done
            for t in range(T):
                sync.wait_ge(ev_sem, NBLK * (t + 1))
                if t >= 1:
                    sync.wait_ge(stw_sem, 16 * t)
                sync.dma_start(
                    out=st_dram.ap()[t], in_=st_sb[t % 2].ap(),
                ).then_inc(stw_sem, 16)
            # phase 2 loads + interleaved output DMAs
            for q in range(4):
                for t in range(T):
                    n = q * T + t
                    if n >= 2:
                        sync.wait_ge(mm_sem, n - 1)
                    sync.wait_ge(stw_sem, 16 * (t + 1))
                    sync.dma_start(
                        out=rb_sb[n % 2].ap(),
                        in_=st_dram.ap()[t, :, q * 1024:(q + 1) * 1024],
                    ).then_inc(rb_sems[n % 2], 16)
                sync.wait_ge(ob_sem, q + 1)
                sync.dma_start(
                    out=out_ext[:, :, q * 1024:(q + 1) * 1024],
                    in_=ob_sb.ap(),
                ).then_inc(out_sem, 16)
            sync.wait_ge(out_sem, 64)

        @block.tensor
        def _(tensor):
            tensor.wait_ge(in_sem, 80)
            for h in range(2):
                if h == 1:
                    tensor.wait_ge(pm_sem, 1)
                for j in range(16):
                    for ch in range(2):
                        mm = tensor.matmul(
                            ps_sh.ap()[:, j, 0:72],
                            xo_sb.ap()[:, ch, (16 * h + j) * 128:(16 * h + j + 1) * 128],
                            wo_sb.ap()[:, ch, :],
                            start=(ch == 0),
                            stop=(ch == 1),
                        )
                        if j == 15 and ch == 1:
                            mm.then_inc(off_sem, 1)
            for t in range(T):
                for i in range(NBLK):
                    g = t * NBLK + i
                    if i == 0:
                        tensor.wait_ge(lerp_sem, 384 * (t + 1))
                    if g >= 2:
                        tensor.wait_ge(ev_sem, g - 1)
                    tensor.transpose(
                        ps_tr[g % 2].ap()[:, 0:128],
                        samp[t % 2].ap()[:, i, :],
                        id_sb.ap(),
                    ).then_inc(tr_sem, 1)
            for q in range(4):
                for t in range(T):
                    n = q * T + t
                    tensor.wait_ge(rb_sems[n % 2], 16 * (n // 2 + 1))
                    if t == 0 and q >= 1:
                        tensor.wait_ge(ob_sem, q)
                    for oc in range(2):
                        for hh in range(2):
                            mm = tensor.matmul(
                                ps_out[:, oc * 2 + hh, :],
                                wd_sb.ap()[:, t, oc * 128:(oc + 1) * 128],
                                rb_sb[n % 2].ap()[:, hh * 512:(hh + 1) * 512],
                                start=(t == 0),
                                stop=(t == 17),
                            )
                            if oc == 1 and hh == 1:
                                mm.then_inc(mm_sem, 1)

        @block.vector
        def _(vector):
            vector.wait_ge(in_sem, 80)
            for h in range(2):
                vector.wait_ge(off_sem, h + 1)
                vector.tensor_tensor(
                    out=p_ap[:, 16 * h:16 * (h + 1), :],
                    in0=ps_sh.ap()[:, :, 0:72],
                    in1=gb_sb.ap()[:, 16 * h:16 * (h + 1), :],
                    op=mybir.AluOpType.add,
                ).then_inc(pm_sem, 1)
            vector.drain()
            vector.tensor_scalar(
                out=fi_ap, in0=p_ap, scalar1=-0.5, scalar2=None,
                op0=mybir.AluOpType.add)
            vector.drain()
            vector.tensor_copy(out=ff_ap, in_=fi_ap)
            vector.drain()
            vector.tensor_tensor(
                out=wf_sb.ap(), in0=p_ap, in1=ff_ap,
                op=mybir.AluOpType.subtract)
            vector.drain()
            vector.tensor_scalar(
                out=ff_ap[:, :, 0:36], in0=ff_ap[:, :, 0:36],
                scalar1=-60.0, scalar2=0.0,
                op0=mybir.AluOpType.add, op1=mybir.AluOpType.max)
            vector.drain()
            vector.tensor_scalar(
                out=ff_ap[:, :, 0:36], in0=ff_ap[:, :, 0:36],
                scalar1=70.0, scalar2=None, op0=mybir.AluOpType.min)
            vector.tensor_scalar(
                out=ff_ap[:, :, 36:72], in0=ff_ap[:, :, 36:72],
                scalar1=-64.0, scalar2=0.0,
                op0=mybir.AluOpType.add, op1=mybir.AluOpType.max)
            vector.drain()
            vector.tensor_scalar(
                out=ff_ap[:, :, 36:72], in0=ff_ap[:, :, 36:72],
                scalar1=71.0, scalar2=None, op0=mybir.AluOpType.min)
            vector.drain()
            vector.scalar_tensor_tensor(
                out=li_sb.ap(), in0=ff_ap[:, :, 0:36], scalar=73.0,
                in1=ff_ap[:, :, 36:72],
                op0=mybir.AluOpType.mult, op1=mybir.AluOpType.add)
            vector.drain()
            vector.tensor_copy(out=li16_sb.ap(), in_=li_sb.ap()).then_inc(idx_sem, 1)
            # hop 2 of idx wrap: idxw[p, t, 8i+u] = idxI[p, u, t, i]
            vector.wait_ge(wrap_sem, 16 * 64)
            vector.drain()
            src = bass.AP(
                tensor=xo_i, offset=0,
                ap=[[16384, 128], [1, 36], [36, 32], [36 * 32, 8]])
            dst = bass.AP(
                tensor=arena2_i, offset=0,
                ap=[[11520, 128], [256, 36], [8, 32], [1, 8]])
            vector.tensor_copy(out=dst, in_=src).then_inc(idx_sem, 1)

            vs = [0]

            def vsync(last):
                last.then_inc(vs_sem, 1)
                vs[0] += 1
                vector.wait_ge(vs_sem, vs[0])

            for t in range(T):
                k2, gp = t // 2, t % 2
                for n_ in (2 * t, 2 * t + 1):
                    vector.wait_ge(g_sems[n_ % 4], 16 * (n_ // 4 + 1))
                if t >= 2:
                    vector.wait_ge(tr_sem, NBLK * (t - 1))
                sp_ = samp[t % 2].ap()
                gaps = [gbuf[(t * 2 + idg) % NGB].ap() for idg in range(2)]
                taps = [(gp * 2 + idg) * 9 + k2 for idg in range(2)]

                def wxb(idg, width):
                    return bass.AP(tensor=wf_sb, offset=36 + taps[idg],
                                   ap=[[2304, 128], [72, 32], [0, width]])

                def wyb(idg, width):
                    return bass.AP(tensor=wf_sb, offset=taps[idg],
                                   ap=[[2304, 128], [72, 32], [0, width]])

                # s0: dpair = [e01|e11] - [e00|e10]  (into slots 128:256)
                last = None
                for idg in range(2):
                    g_ = gaps[idg]
                    last = vector.tensor_tensor(
                        out=g_[:, :, 128:256], in0=g_[:, :, 128:256],
                        in1=g_[:, :, 0:128], op=mybir.AluOpType.subtract)
                vsync(last)
                # s1a: dpair *= wx
                for idg in range(2):
                    g_ = gaps[idg]
                    last = vector.tensor_tensor(
                        out=g_[:, :, 128:256], in0=g_[:, :, 128:256],
                        in1=wxb(idg, 128), op=mybir.AluOpType.mult)
                vsync(last)
                # s1b: upair = dpair + [e00|e10]
                for idg in range(2):
                    g_ = gaps[idg]
                    last = vector.tensor_tensor(
                        out=g_[:, :, 128:256], in0=g_[:, :, 128:256],
                        in1=g_[:, :, 0:128], op=mybir.AluOpType.add)
                vsync(last)
                # s2: dv = u1 - u0 (slots 192:256 -= 128:192)
                for idg in range(2):
                    g_ = gaps[idg]
                    last = vector.tensor_tensor(
                        out=g_[:, :, 192:256], in0=g_[:, :, 192:256],
                        in1=g_[:, :, 128:192], op=mybir.AluOpType.subtract)
                vsync(last)
                # s3a: dv *= wy
                for idg in range(2):
                    g_ = gaps[idg]
                    last = vector.tensor_tensor(
                        out=g_[:, :, 192:256], in0=g_[:, :, 192:256],
                        in1=wyb(idg, 64), op=mybir.AluOpType.mult)
                vsync(last)
                # s3b: samp = dv + u0
                for idg in range(2):
                    g_ = gaps[idg]
                    last = vector.tensor_tensor(
                        out=sp_[:, :, idg * 64:(idg + 1) * 64],
                        in0=g_[:, :, 192:256], in1=g_[:, :, 128:192],
                        op=mybir.AluOpType.add)
                last.then_inc(lerp_sem, 384)

        @block.scalar
        def _(scalar):
            for t in range(T):
                for i in range(NBLK):
                    g = t * NBLK + i
                    scalar.wait_ge(tr_sem, g + 1)
                    if t >= 2 and i == 0:
                        scalar.wait_ge(stw_sem, 16 * (t - 1))
                    scalar.activation(
                        out=st_sb[t % 2].ap()[:, i * 128:(i + 1) * 128],
                        in_=ps_tr[g % 2].ap()[:, 0:128],
                        func=mybir.ActivationFunctionType.Copy,
                    ).then_inc(ev_sem, 1)
            for q in range(4):
                scalar.wait_ge(mm_sem, T * (q + 1))
                if q >= 1:
                    scalar.wait_ge(out_sem, 16 * q)
                scalar.activation(
                    out=ob_sb.ap().rearrange("p a b -> p (a b)"),
                    in_=ps_out.rearrange("p r n -> p (r n)"),
                    func=mybir.ActivationFunctionType.Relu,
                ).then_inc(ob_sem, 1)

        @block.gpsimd
        def _(gpsimd):
            gpsimd.load_library(mlp)
            gpsimd.wait_ge(idx_sem, 2)
            for t in range(T):
                k2, gp = t // 2, t % 2
                for idg in range(2):
                    dg = gp * 2 + idg
                    n = t * 2 + idg
                    if n >= NGB:
                        tm = (n - NGB) // 2
                        gpsimd.wait_ge(lerp_sem, 384 * (tm + 1))
                    tap = dg * 9 + k2
                    gpsimd.dma_gather(
                        out_ap=gbuf[n % NGB].ap(),
                        in_ap=ptab[dg],
                        idxs_ap=idxw_ap[:, tap, :],
                        num_idxs=NPIX,
                        num_idxs_reg=NPIX,
                        elem_size=256,
                        queue_num=n % 4,
                        single_packet=False,
                        prepare_only=True,
                        sem=g_sems[n % 4],
                    ).then_inc(prep_sem, 1)
                    gpsimd.wait_ge(prep_sem, n + 1)
                    gpsimd.trigger_dma(count=1, queue_num=n % 4)


    nc.compile()
    return nc


# ---------------------------------------------------------------------------
# host side
# ---------------------------------------------------------------------------

def _host_prep(x0, x1, x2, w_off, b_off, w_def):
    x = np.concatenate([np.asarray(x0), np.asarray(x1), np.asarray(x2)],
                       axis=1).astype(np.float32)
    w_off = np.asarray(w_off, dtype=np.float32)
    b_off = np.asarray(b_off, dtype=np.float32)
    w_def = np.asarray(w_def, dtype=np.float32)

    perm = np.empty(72, dtype=np.int64)
    for dg in range(DG):
        for k2 in range(K2):
            perm[dg * 9 + k2] = (dg * 9 + k2) * 2
            perm[36 + dg * 9 + k2] = (dg * 9 + k2) * 2 + 1
    w_off_perm = w_off[perm]
    b_off_perm = b_off[perm]

    ky = np.repeat(np.arange(3), 3).astype(np.float32)
    kx = np.tile(np.arange(3), 3).astype(np.float32)

    w_off_p = np.empty((128, 2, 72), dtype=np.float32)
    for ch in range(2):
        w_off_p[:, ch, :] = w_off_perm[:, ch * 128:(ch + 1) * 128].T

    pix = np.arange(32)[None, :] * 128 + np.arange(128)[:, None]
    hh = (pix // WB).astype(np.float32)
    wl = (pix % WB).astype(np.float32)
    gridbias = np.empty((128, 32, 72), dtype=np.float32)
    for dg in range(DG):
        for k2 in range(K2):
            o = dg * 9 + k2
            gridbias[:, :, o] = hh + (ky[k2] + 63.0 + b_off_perm[o])
            gridbias[:, :, 36 + o] = wl + (kx[k2] + 67.0 + b_off_perm[36 + o])

    w_def_p = np.empty((128, 18, 256), dtype=np.float32)
    for k2 in range(K2):
        for gp in range(2):
            w_def_p[:, k2 * 2 + gp, :] = w_def[:, gp * 128:(gp + 1) * 128, k2].T
    w_def_p = w_def_p.astype(bf16)

    ident = np.eye(128, dtype=np.float32).astype(bf16)

    in_maps = []
    for core in range(8):
        b, wb = core // 4, core % 4
        ws = wb * WB
        strip = np.zeros((C, HS, TW + 1), dtype=np.float32)
        lo, hi = ws - PAD, ws - PAD + TW + 1
        slo, shi = max(0, lo), min(W, hi)
        strip[:, PAD:PAD + H, slo - lo:shi - lo] = x[b, :, :, slo:shi]
        strip_bf = strip.astype(bf16)

        ptab = np.zeros((DG, HS, TW, 2, 2, CG), dtype=bf16)
        for dg in range(DG):
            sc = np.moveaxis(strip_bf[dg * CG:(dg + 1) * CG], 0, -1)
            scp = np.zeros((HS + 1, TW + 1, CG), dtype=bf16)
            scp[:HS] = sc
            ptab[dg, :, :, 0, 0] = scp[:HS, :TW]        # e00
            ptab[dg, :, :, 0, 1] = scp[1:HS + 1, :TW]    # e10
            ptab[dg, :, :, 1, 0] = scp[:HS, 1:TW + 1]    # e01
            ptab[dg, :, :, 1, 1] = scp[1:HS + 1, 1:TW + 1]  # e11
        ptab = ptab.reshape(DG, NROWS, 256)

        xo = x[b, :, :, ws:ws + WB].reshape(C, NPIX)
        x_off = np.empty((128, 2, NPIX), dtype=np.float32)
        for ch in range(2):
            x_off[:, ch, :] = xo[ch * 128:(ch + 1) * 128]

        in_maps.append({
            "x_off": np.ascontiguousarray(x_off),
            "ptab": np.ascontiguousarray(ptab),
            "w_off_p": np.ascontiguousarray(w_off_p),
            "gridbias": np.ascontiguousarray(gridbias),
            "w_def_p": np.ascontiguousarray(w_def_p),
            "ident": ident,
        })
    return in_maps


def _assemble(outs):
    full = np.empty((B, C_OUT, H, W), dtype=np.float32)
    for core in range(8):
        b, wb = core // 4, core % 4
        o = np.asarray(outs[core])
        for oc in range(2):
            full[b, oc * 128:(oc + 1) * 128, :, wb * WB:(wb + 1) * WB] = \
                o[:, oc, :].reshape(128, H, WB)
    return full


def kernel(**inputs):
    sys.path.insert(0, "/root/problem")
    try:
        import axon_shim  # noqa: F401  (enables NTFF profiling when present)
    except Exception:
        pass
    from concourse.bass_utils import run_bass_kernel_spmd

    in_maps = _host_prep(**inputs)
    if "nc" not in _GRAPH_CACHE:
        _GRAPH_CACHE["nc"] = build_graph()
    nc = _GRAPH_CACHE["nc"]
    res = run_bass_kernel_spmd(nc, in_maps, list(range(8)))
    return _assemble([r["out"] for r in res.results])


# revision 19
# speedup vs baseline: 1.0392x; 1.0392x over previous
"""Deformable-conv fusion (nn_AdaptionFusion) on 8 TRN2 NeuronCores.

Data-parallel: core i handles (batch i//4, 64-col W-block i%4); weights
replicated; no collectives. See bottom for kernel() entry point.

Device pipeline per core:
  offset conv (PE, 2 psum half-passes) -> index math (DVE) ->
  idx wrap (64 fold-DMAs) -> 36x dma_gather of 2x2-pixel patch rows
  (bf16, 512B) -> bilinear lerp (DVE scalar_tensor_tensor, per-partition
  fracs) -> PE transpose to (ck, pix) -> staged to DRAM ->
  K=2304 matmul in 18 chunks (PE) -> ReLU (ACT) -> out.
"""

import sys
sys.path.insert(0, "/opt/trn_rl_repo")
import numpy as np
import ml_dtypes

import concourse.bass as bass
from concourse import bacc, mybir
from concourse.library_config import mlp

B, H, W = 2, 64, 256
C, DG, CG, K2, C_OUT = 256, 4, 64, 9, 256
WB, PAD = 64, 4
HS, WS = H + 2 * PAD, WB + 2 * PAD
TW = WS + 1
NPIX = H * WB
NROWS = HS * TW
T = 18
NBLK = 32
NGB = 4
bf16 = ml_dtypes.bfloat16

_GRAPH_CACHE = {}


def build_graph():
    f32, i16, bf = mybir.dt.float32, mybir.dt.int16, mybir.dt.bfloat16
    nc = bacc.Bacc("TRN2", target_bir_lowering=False, num_swdge_queues=4)

    x_off = nc.declare_dram_parameter("x_off", [128, 2, NPIX], f32, isOutput=False)
    ptab = nc.declare_dram_parameter("ptab", [DG, NROWS, 256], bf, isOutput=False)
    w_off_p = nc.declare_dram_parameter("w_off_p", [128, 2, 72], f32, isOutput=False)
    gridbias = nc.declare_dram_parameter("gridbias", [128, 32, 72], f32, isOutput=False)
    w_def_p = nc.declare_dram_parameter("w_def_p", [128, 18, 256], bf, isOutput=False)
    ident_e = nc.declare_dram_parameter("ident", [128, 128], bf, isOutput=False)
    out_ext = nc.declare_dram_parameter("out", [128, 2, NPIX], f32, isOutput=True)
    st_dram = nc.dram_tensor("st_dram", [T, 128, NPIX], bf)

    xo_sb = nc.alloc_sbuf_tensor("xo_sb", [128, 2, NPIX], f32)
    wo_sb = nc.alloc_sbuf_tensor("wo_sb", [128, 2, 72], f32)
    gb_sb = nc.alloc_sbuf_tensor("gb_sb", [128, 32, 72], f32)
    wd_sb = nc.alloc_sbuf_tensor("wd_sb", [128, 18, 256], bf)
    id_sb = nc.alloc_sbuf_tensor("id_sb", [128, 128], bf)
    arena2 = nc.alloc_sbuf_tensor("arena2", [128, 5760], f32)
    arena2_i = arena2.bitcast(i16)          # pitch 11520 i16
    xo_i = xo_sb.bitcast(i16)               # pitch 16384 i16
    wf_sb = nc.alloc_sbuf_tensor("wf_sb", [128, 32, 72], bf)
    li_sb = nc.alloc_sbuf_tensor("li_sb", [128, 32, 36], f32)
    li16_sb = nc.alloc_sbuf_tensor("li16_sb", [128, 32, 36], i16)

    def _v(tensor, pitch, off, dims):
        return bass.AP(tensor=tensor, offset=off, ap=[[pitch, 128]] + dims)

    # overlays: p/fi/ff share arena2 with idxw (idxw written after all dead)
    p_ap = _v(arena2, 5760, 0, [[72, 32], [1, 72]])
    fi_ap = _v(arena2_i, 11520, 4608, [[72, 32], [1, 72]])
    ff_ap = _v(arena2, 5760, 3456, [[72, 32], [1, 72]])
    idxw_ap = _v(arena2_i, 11520, 0, [[256, 36], [1, 256]])
    # idxI overlays xo (xo dead after offset conv)
    idxI_ap = _v(xo_i, 16384, 0, [[36 * 32, 8], [32, 36], [1, 32]])
    tmp_sb = nc.alloc_sbuf_tensor("tmp_sb", [128, 128], bf)
    gbuf = [nc.alloc_sbuf_tensor(f"gbuf{i}", [128, 32, 256], bf) for i in range(NGB)]
    samp = [nc.alloc_sbuf_tensor(f"samp{i}", [128, 32, 128], bf) for i in range(2)]
    # st buffers overlaid on dead regions: st0 over gb_sb (dead after p-add),
    # st1 over the xo arena tail (idxI uses [0,18432B); st1 at byte 18432)
    rb_sb = [nc.alloc_sbuf_tensor(f"rb{i}", [128, 1024], bf) for i in range(2)]
    ob_sb = nc.alloc_sbuf_tensor("ob_sb", [128, 2, 1024], f32)

    gb_bf = gb_sb.bitcast(bf)
    st_aps = [
        bass.AP(tensor=gb_bf, offset=0, ap=[[4608, 128], [1, NPIX]]),
        bass.AP(tensor=xo_i.bitcast(bf) if False else xo_sb.bitcast(bf), offset=9216,
                ap=[[16384, 128], [1, NPIX]]),
    ]

    ps_sh = nc.alloc_psum_tensor("ps_sh", [128, 16, 128], f32)   # 8KB banks 0-3
    ps_tr = [nc.alloc_psum_tensor(f"ps_tr{i}", [128, 1024], bf) for i in range(2)]
    ps_out = ps_sh.ap().rearrange("p a b -> p (a b)").rearrange(
        "p (r n) -> p r n", r=4)                                  # [128, 4, 512] view

    with (
        nc.Block() as block,
        nc.semaphore("in_sem") as in_sem,
        nc.semaphore("off_sem") as off_sem,
        nc.semaphore("pm_sem") as pm_sem,
        nc.semaphore("idx_sem") as idx_sem,
        nc.semaphore("wrap_sem") as wrap_sem,
        nc.semaphore("g_sem0") as g_sem0,
        nc.semaphore("g_sem1") as g_sem1,
        nc.semaphore("g_sem2") as g_sem2,
        nc.semaphore("g_sem3") as g_sem3,
        nc.semaphore("lerp_sem") as lerp_sem,
        nc.semaphore("tr_sem") as tr_sem,
        nc.semaphore("ev_sem") as ev_sem,
        nc.semaphore("stw_sem") as stw_sem,
        nc.semaphore("rb_sem0") as rb_sem0,
        nc.semaphore("rb_sem1") as rb_sem1,
        nc.semaphore("mm_sem") as mm_sem,
        nc.semaphore("prep_sem") as prep_sem,
        nc.semaphore("vs_sem") as vs_sem,
        nc.semaphore("ob_sem") as ob_sem,
        nc.semaphore("out_sem") as out_sem,
    ):
        g_sems = [g_sem0, g_sem1, g_sem2, g_sem3]
        rb_sems = [rb_sem0, rb_sem1]

        @block.sync
        def _(sync):
            sync.dma_start(out=xo_sb.ap(), in_=x_off[:]).then_inc(in_sem, 16)
            sync.dma_start(out=wo_sb.ap(), in_=w_off_p[:]).then_inc(in_sem, 16)
            sync.dma_start(out=gb_sb.ap(), in_=gridbias[:]).then_inc(in_sem, 16)
            sync.dma_start(out=wd_sb.ap(), in_=w_def_p[:]).then_inc(in_sem, 16)
            sync.dma_start(out=id_sb.ap(), in_=ident_e[:]).then_inc(in_sem, 16)
            # idx wrap: idxw[16c+q, t, 8i+u] = li16[16u+q, i, t]
            sync.wait_ge(idx_sem, 1)
            # hop 1: idxI[16c+q, u, t, i] = li16[16u+q, i, t]
            for c in range(8):
                for u in range(8):
                    src = bass.AP(
                        tensor=li16_sb, offset=(16 * u) * (32 * 36),
                        ap=[[32 * 36, 16], [36, 32], [1, 36]])
                    dst = bass.AP(
                        tensor=xo_i, offset=(16 * c) * 16384 + u * (36 * 32),
                        ap=[[16384, 16], [36, 32], [1, 36]])
                    sync.dma_start(out=dst, in_=src).then_inc(wrap_sem, 16)
            # st writes (HWDGE): st[t] once its evictions are done
            for t in range(T):
                sync.wait_ge(ev_sem, NBLK * (t + 1))
                if t >= 1:
                    sync.wait_ge(stw_sem, 16 * t)
                sync.dma_start(
                    out=st_dram.ap()[t], in_=st_aps[t % 2],
                ).then_inc(stw_sem, 16)
            # phase 2 loads + interleaved output DMAs
            for q in range(4):
                for t in range(T):
                    n = q * T + t
                    if n >= 2:
                        sync.wait_ge(mm_sem, n - 1)
                    sync.wait_ge(stw_sem, 16 * (t + 1))
                    sync.dma_start(
                        out=rb_sb[n % 2].ap(),
                        in_=st_dram.ap()[t, :, q * 1024:(q + 1) * 1024],
                    ).then_inc(rb_sems[n % 2], 16)
                sync.wait_ge(ob_sem, q + 1)
                sync.dma_start(
                    out=out_ext[:, :, q * 1024:(q + 1) * 1024],
                    in_=ob_sb.ap(),
                ).then_inc(out_sem, 16)
            sync.wait_ge(out_sem, 64)

        @block.tensor
        def _(tensor):
            tensor.wait_ge(in_sem, 80)
            for h in range(2):
                if h == 1:
                    tensor.wait_ge(pm_sem, 1)
                for j in range(16):
                    for ch in range(2):
                        mm = tensor.matmul(
                            ps_sh.ap()[:, j, 0:72],
                            xo_sb.ap()[:, ch, (16 * h + j) * 128:(16 * h + j + 1) * 128],
                            wo_sb.ap()[:, ch, :],
                            start=(ch == 0),
                            stop=(ch == 1),
                        )
                        if j == 15 and ch == 1:
                            mm.then_inc(off_sem, 1)
            for t in range(T):
                for i in range(NBLK):
                    g = t * NBLK + i
                    if i == 0:
                        tensor.wait_ge(lerp_sem, 384 * (t + 1))
                    if g >= 2:
                        tensor.wait_ge(ev_sem, g - 1)
                    tensor.transpose(
                        ps_tr[g % 2].ap()[:, 0:128],
                        samp[t % 2].ap()[:, i, :],
                        id_sb.ap(),
                    ).then_inc(tr_sem, 1)
            for q in range(4):
                for t in range(T):
                    n = q * T + t
                    tensor.wait_ge(rb_sems[n % 2], 16 * (n // 2 + 1))
                    if t == 0 and q >= 1:
                        tensor.wait_ge(ob_sem, q)
                    for oc in range(2):
                        for hh in range(2):
                            mm = tensor.matmul(
                                ps_out[:, oc * 2 + hh, :],
                                wd_sb.ap()[:, t, oc * 128:(oc + 1) * 128],
                                rb_sb[n % 2].ap()[:, hh * 512:(hh + 1) * 512],
                                start=(t == 0),
                                stop=(t == 17),
                            )
                            if oc == 1 and hh == 1:
                                mm.then_inc(mm_sem, 1)

        @block.vector
        def _(vector):
            vector.wait_ge(in_sem, 80)
            for h in range(2):
                vector.wait_ge(off_sem, h + 1)
                vector.tensor_tensor(
                    out=p_ap[:, 16 * h:16 * (h + 1), :],
                    in0=ps_sh.ap()[:, :, 0:72],
                    in1=gb_sb.ap()[:, 16 * h:16 * (h + 1), :],
                    op=mybir.AluOpType.add,
                ).then_inc(pm_sem, 1)
            vector.drain()
            vector.tensor_scalar(
                out=fi_ap, in0=p_ap, scalar1=-0.5, scalar2=None,
                op0=mybir.AluOpType.add)
            vector.drain()
            vector.tensor_copy(out=ff_ap, in_=fi_ap)
            vector.drain()
            vector.tensor_tensor(
                out=wf_sb.ap(), in0=p_ap, in1=ff_ap,
                op=mybir.AluOpType.subtract)
            vector.drain()
            vector.tensor_scalar(
                out=ff_ap[:, :, 0:36], in0=ff_ap[:, :, 0:36],
                scalar1=-60.0, scalar2=0.0,
                op0=mybir.AluOpType.add, op1=mybir.AluOpType.max)
            vector.drain()
            vector.tensor_scalar(
                out=ff_ap[:, :, 0:36], in0=ff_ap[:, :, 0:36],
                scalar1=70.0, scalar2=None, op0=mybir.AluOpType.min)
            vector.tensor_scalar(
                out=ff_ap[:, :, 36:72], in0=ff_ap[:, :, 36:72],
                scalar1=-64.0, scalar2=0.0,
                op0=mybir.AluOpType.add, op1=mybir.AluOpType.max)
            vector.drain()
            vector.tensor_scalar(
                out=ff_ap[:, :, 36:72], in0=ff_ap[:, :, 36:72],
                scalar1=71.0, scalar2=None, op0=mybir.AluOpType.min)
            vector.drain()
            vector.scalar_tensor_tensor(
                out=li_sb.ap(), in0=ff_ap[:, :, 0:36], scalar=73.0,
                in1=ff_ap[:, :, 36:72],
                op0=mybir.AluOpType.mult, op1=mybir.AluOpType.add)
            vector.drain()
            vector.tensor_copy(out=li16_sb.ap(), in_=li_sb.ap()).then_inc(idx_sem, 1)
            # hop 2 of idx wrap: idxw[p, t, 8i+u] = idxI[p, u, t, i]
            vector.wait_ge(wrap_sem, 16 * 64)
            vector.drain()
            src = bass.AP(
                tensor=xo_i, offset=0,
                ap=[[16384, 128], [1, 36], [36, 32], [36 * 32, 8]])
            dst = bass.AP(
                tensor=arena2_i, offset=0,
                ap=[[11520, 128], [256, 36], [8, 32], [1, 8]])
            vector.tensor_copy(out=dst, in_=src).then_inc(idx_sem, 1)

            vs = [0]

            def vsync(last):
                last.then_inc(vs_sem, 1)
                vs[0] += 1
                vector.wait_ge(vs_sem, vs[0])

            for t in range(T):
                k2, gp = t // 2, t % 2
                for n_ in (2 * t, 2 * t + 1):
                    vector.wait_ge(g_sems[n_ % 4], 16 * (n_ // 4 + 1))
                if t >= 2:
                    vector.wait_ge(tr_sem, NBLK * (t - 1))
                sp_ = samp[t % 2].ap()
                gaps = [gbuf[(t * 2 + idg) % NGB].ap() for idg in range(2)]
                taps = [(gp * 2 + idg) * 9 + k2 for idg in range(2)]

                def wxb(idg, width):
                    return bass.AP(tensor=wf_sb, offset=36 + taps[idg],
                                   ap=[[2304, 128], [72, 32], [0, width]])

                def wyb(idg, width):
                    return bass.AP(tensor=wf_sb, offset=taps[idg],
                                   ap=[[2304, 128], [72, 32], [0, width]])

                # s0: dpair = [e01|e11] - [e00|e10]  (into slots 128:256)
                last = None
                for idg in range(2):
                    g_ = gaps[idg]
                    last = vector.tensor_tensor(
                        out=g_[:, :, 128:256], in0=g_[:, :, 128:256],
                        in1=g_[:, :, 0:128], op=mybir.AluOpType.subtract)
                vsync(last)
                # s1a: dpair *= wx
                for idg in range(2):
                    g_ = gaps[idg]
                    last = vector.tensor_tensor(
                        out=g_[:, :, 128:256], in0=g_[:, :, 128:256],
                        in1=wxb(idg, 128), op=mybir.AluOpType.mult)
                vsync(last)
                # s1b: upair = dpair + [e00|e10]
                for idg in range(2):
                    g_ = gaps[idg]
                    last = vector.tensor_tensor(
                        out=g_[:, :, 128:256], in0=g_[:, :, 128:256],
                        in1=g_[:, :, 0:128], op=mybir.AluOpType.add)
                vsync(last)
                # s2: dv = u1 - u0 (slots 192:256 -= 128:192)
                for idg in range(2):
                    g_ = gaps[idg]
                    last = vector.tensor_tensor(
                        out=g_[:, :, 192:256], in0=g_[:, :, 192:256],
                        in1=g_[:, :, 128:192], op=mybir.AluOpType.subtract)
                vsync(last)
                # s3a: dv *= wy
                for idg in range(2):
                    g_ = gaps[idg]
                    last = vector.tensor_tensor(
                        out=g_[:, :, 192:256], in0=g_[:, :, 192:256],
                        in1=wyb(idg, 64), op=mybir.AluOpType.mult)
                vsync(last)
                # s3b: samp = dv + u0
                for idg in range(2):
                    g_ = gaps[idg]
                    last = vector.tensor_tensor(
                        out=sp_[:, :, idg * 64:(idg + 1) * 64],
                        in0=g_[:, :, 192:256], in1=g_[:, :, 128:192],
                        op=mybir.AluOpType.add)
                last.then_inc(lerp_sem, 384)

        @block.scalar
        def _(scalar):
            for t in range(T):
                for i in range(NBLK):
                    g = t * NBLK + i
                    scalar.wait_ge(tr_sem, g + 1)
                    if t >= 2 and i == 0:
                        scalar.wait_ge(stw_sem, 16 * (t - 1))
                    scalar.activation(
                        out=st_aps[t % 2][:, i * 128:(i + 1) * 128],
                        in_=ps_tr[g % 2].ap()[:, 0:128],
                        func=mybir.ActivationFunctionType.Copy,
                    ).then_inc(ev_sem, 1)
            for q in range(4):
                scalar.wait_ge(mm_sem, T * (q + 1))
                if q >= 1:
                    scalar.wait_ge(out_sem, 16 * q)
                scalar.activation(
                    out=ob_sb.ap().rearrange("p a b -> p (a b)"),
                    in_=ps_out.rearrange("p r n -> p (r n)"),
                    func=mybir.ActivationFunctionType.Relu,
                ).then_inc(ob_sem, 1)

        @block.gpsimd
        def _(gpsimd):
            gpsimd.load_library(mlp)
            gpsimd.wait_ge(idx_sem, 2)
            for t in range(T):
                k2, gp = t // 2, t % 2
                for idg in range(2):
                    dg = gp * 2 + idg
                    n = t * 2 + idg
                    if n >= NGB:
                        tm = (n - NGB) // 2
                        gpsimd.wait_ge(lerp_sem, 384 * (tm + 1))
                    tap = dg * 9 + k2
                    gpsimd.dma_gather(
                        out_ap=gbuf[n % NGB].ap(),
                        in_ap=ptab[dg],
                        idxs_ap=idxw_ap[:, tap, :],
                        num_idxs=NPIX,
                        num_idxs_reg=NPIX,
                        elem_size=256,
                        queue_num=n % 4,
                        single_packet=False,
                        prepare_only=True,
                        sem=g_sems[n % 4],
                    ).then_inc(prep_sem, 1)
                    gpsimd.wait_ge(prep_sem, n + 1)
                    gpsimd.trigger_dma(count=1, queue_num=n % 4)


    nc.compile()
    return nc


# ---------------------------------------------------------------------------
# host side
# ---------------------------------------------------------------------------

def _host_prep(x0, x1, x2, w_off, b_off, w_def):
    x = np.concatenate([np.asarray(x0), np.asarray(x1), np.asarray(x2)],
                       axis=1).astype(np.float32)
    w_off = np.asarray(w_off, dtype=np.float32)
    b_off = np.asarray(b_off, dtype=np.float32)
    w_def = np.asarray(w_def, dtype=np.float32)

    perm = np.empty(72, dtype=np.int64)
    for dg in range(DG):
        for k2 in range(K2):
            perm[dg * 9 + k2] = (dg * 9 + k2) * 2
            perm[36 + dg * 9 + k2] = (dg * 9 + k2) * 2 + 1
    w_off_perm = w_off[perm]
    b_off_perm = b_off[perm]

    ky = np.repeat(np.arange(3), 3).astype(np.float32)
    kx = np.tile(np.arange(3), 3).astype(np.float32)

    w_off_p = np.empty((128, 2, 72), dtype=np.float32)
    for ch in range(2):
        w_off_p[:, ch, :] = w_off_perm[:, ch * 128:(ch + 1) * 128].T

    pix = np.arange(32)[None, :] * 128 + np.arange(128)[:, None]
    hh = (pix // WB).astype(np.float32)
    wl = (pix % WB).astype(np.float32)
    gridbias = np.empty((128, 32, 72), dtype=np.float32)
    for dg in range(DG):
        for k2 in range(K2):
            o = dg * 9 + k2
            gridbias[:, :, o] = hh + (ky[k2] + 63.0 + b_off_perm[o])
            gridbias[:, :, 36 + o] = wl + (kx[k2] + 67.0 + b_off_perm[36 + o])

    w_def_p = np.empty((128, 18, 256), dtype=np.float32)
    for k2 in range(K2):
        for gp in range(2):
            w_def_p[:, k2 * 2 + gp, :] = w_def[:, gp * 128:(gp + 1) * 128, k2].T
    w_def_p = w_def_p.astype(bf16)

    ident = np.eye(128, dtype=np.float32).astype(bf16)

    in_maps = []
    for core in range(8):
        b, wb = core // 4, core % 4
        ws = wb * WB
        strip = np.zeros((C, HS, TW + 1), dtype=np.float32)
        lo, hi = ws - PAD, ws - PAD + TW + 1
        slo, shi = max(0, lo), min(W, hi)
        strip[:, PAD:PAD + H, slo - lo:shi - lo] = x[b, :, :, slo:shi]
        strip_bf = strip.astype(bf16)

        ptab = np.zeros((DG, HS, TW, 2, 2, CG), dtype=bf16)
        for dg in range(DG):
            sc = np.moveaxis(strip_bf[dg * CG:(dg + 1) * CG], 0, -1)
            scp = np.zeros((HS + 1, TW + 1, CG), dtype=bf16)
            scp[:HS] = sc
            ptab[dg, :, :, 0, 0] = scp[:HS, :TW]        # e00
            ptab[dg, :, :, 0, 1] = scp[1:HS + 1, :TW]    # e10
            ptab[dg, :, :, 1, 0] = scp[:HS, 1:TW + 1]    # e01
            ptab[dg, :, :, 1, 1] = scp[1:HS + 1, 1:TW + 1]  # e11
        ptab = ptab.reshape(DG, NROWS, 256)

        xo = x[b, :, :, ws:ws + WB].reshape(C, NPIX)
        x_off = np.empty((128, 2, NPIX), dtype=np.float32)
        for ch in range(2):
            x_off[:, ch, :] = xo[ch * 128:(ch + 1) * 128]

        in_maps.append({
            "x_off": np.ascontiguousarray(x_off),
            "ptab": np.ascontiguousarray(ptab),
            "w_off_p": np.ascontiguousarray(w_off_p),
            "gridbias": np.ascontiguousarray(gridbias),
            "w_def_p": np.ascontiguousarray(w_def_p),
            "ident": ident,
        })
    return in_maps


def _assemble(outs):
    full = np.empty((B, C_OUT, H, W), dtype=np.float32)
    for core in range(8):
        b, wb = core // 4, core % 4
        o = np.asarray(outs[core])
        for oc in range(2):
            full[b, oc * 128:(oc + 1) * 128, :, wb * WB:(wb + 1) * WB] = \
                o[:, oc, :].reshape(128, H, WB)
    return full


def kernel(**inputs):
    sys.path.insert(0, "/root/problem")
    try:
        import axon_shim  # noqa: F401  (enables NTFF profiling when present)
    except Exception:
        pass
    from concourse.bass_utils import run_bass_kernel_spmd

    in_maps = _host_prep(**inputs)
    if "nc" not in _GRAPH_CACHE:
        _GRAPH_CACHE["nc"] = build_graph()
    nc = _GRAPH_CACHE["nc"]
    res = run_bass_kernel_spmd(nc, in_maps, list(range(8)))
    return _assemble([r["out"] for r in res.results])


# revision 20
# speedup vs baseline: 1.4594x; 1.4043x over previous
"""Deformable-conv fusion (nn_AdaptionFusion) on 8 TRN2 NeuronCores.

Data-parallel: core i handles (batch i//4, 64-col W-block i%4); weights
replicated; no collectives. See bottom for kernel() entry point.

Device pipeline per core:
  offset conv (PE, 2 psum half-passes) -> index math (DVE) ->
  idx wrap (64 fold-DMAs) -> 36x dma_gather of 2x2-pixel patch rows
  (bf16, 512B) -> bilinear lerp (DVE scalar_tensor_tensor, per-partition
  fracs) -> PE transpose to (ck, pix) -> staged to DRAM ->
  K=2304 matmul in 18 chunks (PE) -> ReLU (ACT) -> out.
"""

import sys
sys.path.insert(0, "/opt/trn_rl_repo")
import numpy as np
import ml_dtypes

import concourse.bass as bass
from concourse import bacc, mybir
from concourse.library_config import mlp

B, H, W = 2, 64, 256
C, DG, CG, K2, C_OUT = 256, 4, 64, 9, 256
WB, PAD = 64, 4
HS, WS = H + 2 * PAD, WB + 2 * PAD
TW = WS + 1
NPIX = H * WB
NROWS = HS * TW
T = 18
NBLK = 32
NGB = 4
bf16 = ml_dtypes.bfloat16

_GRAPH_CACHE = {}


def build_graph():
    f32, i16, bf = mybir.dt.float32, mybir.dt.int16, mybir.dt.bfloat16
    nc = bacc.Bacc("TRN2", target_bir_lowering=False, num_swdge_queues=4)

    x_off = nc.declare_dram_parameter("x_off", [128, 2, NPIX], f32, isOutput=False)
    ptab = nc.declare_dram_parameter("ptab", [DG, NROWS, 256], bf, isOutput=False)
    w_off_p = nc.declare_dram_parameter("w_off_p", [128, 2, 72], f32, isOutput=False)
    gridbias = nc.declare_dram_parameter("gridbias", [128, 32, 72], f32, isOutput=False)
    w_def_p = nc.declare_dram_parameter("w_def_p", [128, 18, 256], bf, isOutput=False)
    ident_e = nc.declare_dram_parameter("ident", [128, 128], bf, isOutput=False)
    out_ext = nc.declare_dram_parameter("out", [128, 2, NPIX], f32, isOutput=True)
    st_dram = nc.dram_tensor("st_dram", [T, 128, NPIX], bf)

    xo_sb = nc.alloc_sbuf_tensor("xo_sb", [128, 2, NPIX], f32)
    wo_sb = nc.alloc_sbuf_tensor("wo_sb", [128, 2, 72], f32)
    gb_sb = nc.alloc_sbuf_tensor("gb_sb", [128, 32, 72], f32)
    wd_sb = nc.alloc_sbuf_tensor("wd_sb", [128, 18, 256], bf)
    id_sb = nc.alloc_sbuf_tensor("id_sb", [128, 128], bf)
    arena2 = nc.alloc_sbuf_tensor("arena2", [128, 5760], f32)
    arena2_i = arena2.bitcast(i16)          # pitch 11520 i16
    xo_i = xo_sb.bitcast(i16)               # pitch 16384 i16
    wf_sb = nc.alloc_sbuf_tensor("wf_sb", [128, 32, 72], bf)
    li_sb = nc.alloc_sbuf_tensor("li_sb", [128, 32, 36], f32)
    li16_sb = nc.alloc_sbuf_tensor("li16_sb", [128, 32, 36], i16)

    def _v(tensor, pitch, off, dims):
        return bass.AP(tensor=tensor, offset=off, ap=[[pitch, 128]] + dims)

    # overlays: p/fi/ff share arena2 with idxw (idxw written after all dead)
    p_ap = _v(arena2, 5760, 0, [[72, 32], [1, 72]])
    fi_ap = _v(arena2_i, 11520, 4608, [[72, 32], [1, 72]])
    ff_ap = _v(arena2, 5760, 3456, [[72, 32], [1, 72]])
    idxw_ap = _v(arena2_i, 11520, 0, [[256, 36], [1, 256]])
    # idxI overlays xo (xo dead after offset conv)
    idxI_ap = _v(xo_i, 16384, 0, [[36 * 32, 8], [32, 36], [1, 32]])
    tmp_sb = nc.alloc_sbuf_tensor("tmp_sb", [128, 128], bf)
    gbuf = [nc.alloc_sbuf_tensor(f"gbuf{i}", [128, 32, 256], bf) for i in range(NGB)]
    samp = [nc.alloc_sbuf_tensor(f"samp{i}", [128, 32, 128], bf) for i in range(2)]
    # st buffers overlaid on dead regions: st0 over gb_sb (dead after p-add),
    # st1 over the xo arena tail (idxI uses [0,18432B); st1 at byte 18432)
    rb_sb = [nc.alloc_sbuf_tensor(f"rb{i}", [128, 1024], bf) for i in range(2)]
    ob_sb = nc.alloc_sbuf_tensor("ob_sb", [128, 2, 1024], f32)

    gb_bf = gb_sb.bitcast(bf)
    st_aps = [
        bass.AP(tensor=gb_bf, offset=0, ap=[[4608, 128], [1, NPIX]]),
        bass.AP(tensor=xo_i.bitcast(bf) if False else xo_sb.bitcast(bf), offset=9216,
                ap=[[16384, 128], [1, NPIX]]),
    ]

    ps_sh = nc.alloc_psum_tensor("ps_sh", [128, 16, 128], f32)   # 8KB banks 0-3
    ps_tr = [nc.alloc_psum_tensor(f"ps_tr{i}", [128, 1024], bf) for i in range(2)]
    ps_out = ps_sh.ap().rearrange("p a b -> p (a b)").rearrange(
        "p (r n) -> p r n", r=4)                                  # [128, 4, 512] view

    with (
        nc.Block() as block,
        nc.semaphore("in_sem") as in_sem,
        nc.semaphore("off_sem") as off_sem,
        nc.semaphore("pm_sem") as pm_sem,
        nc.semaphore("idx_sem") as idx_sem,
        nc.semaphore("wrap_sem") as wrap_sem,
        nc.semaphore("g_sem0") as g_sem0,
        nc.semaphore("g_sem1") as g_sem1,
        nc.semaphore("g_sem2") as g_sem2,
        nc.semaphore("g_sem3") as g_sem3,
        nc.semaphore("lerp_sem") as lerp_sem,
        nc.semaphore("tr_sem") as tr_sem,
        nc.semaphore("ev_sem") as ev_sem,
        nc.semaphore("stw_sem") as stw_sem,
        nc.semaphore("rb_sem0") as rb_sem0,
        nc.semaphore("rb_sem1") as rb_sem1,
        nc.semaphore("mm_sem") as mm_sem,
        nc.semaphore("prep_sem") as prep_sem,
        nc.semaphore("vs_sem") as vs_sem,
        nc.semaphore("ob_sem") as ob_sem,
        nc.semaphore("out_sem") as out_sem,
    ):
        g_sems = [g_sem0, g_sem1, g_sem2, g_sem3]
        rb_sems = [rb_sem0, rb_sem1]

        @block.sync
        def _(sync):
            sync.dma_start(out=xo_sb.ap(), in_=x_off[:]).then_inc(in_sem, 16)
            sync.dma_start(out=wo_sb.ap(), in_=w_off_p[:]).then_inc(in_sem, 16)
            sync.dma_start(out=gb_sb.ap(), in_=gridbias[:]).then_inc(in_sem, 16)
            sync.dma_start(out=wd_sb.ap(), in_=w_def_p[:]).then_inc(in_sem, 16)
            sync.dma_start(out=id_sb.ap(), in_=ident_e[:]).then_inc(in_sem, 16)
            # idx wrap: idxw[16c+q, t, 8i+u] = li16[16u+q, i, t]
            sync.wait_ge(idx_sem, 1)
            # hop 1: idxI[16c+q, u, t, i] = li16[16u+q, i, t]
            for c in range(8):
                for u in range(8):
                    src = bass.AP(
                        tensor=li16_sb, offset=(16 * u) * (32 * 36),
                        ap=[[32 * 36, 16], [36, 32], [1, 36]])
                    dst = bass.AP(
                        tensor=xo_i, offset=(16 * c) * 16384 + u * (36 * 32),
                        ap=[[16384, 16], [36, 32], [1, 36]])
                    sync.dma_start(out=dst, in_=src).then_inc(wrap_sem, 16)
            # st writes (HWDGE): st[t] once its evictions are done
            for t in range(T):
                sync.wait_ge(ev_sem, NBLK * (t + 1))
                if t >= 1:
                    sync.wait_ge(stw_sem, 16 * t)
                sync.dma_start(
                    out=st_dram.ap()[t], in_=st_aps[t % 2],
                ).then_inc(stw_sem, 16)
            # phase 2 loads + interleaved output DMAs
            for q in range(4):
                for t in range(T):
                    n = q * T + t
                    if n >= 2:
                        sync.wait_ge(mm_sem, n - 1)
                    sync.wait_ge(stw_sem, 16 * (t + 1))
                    sync.dma_start(
                        out=rb_sb[n % 2].ap(),
                        in_=st_dram.ap()[t, :, q * 1024:(q + 1) * 1024],
                    ).then_inc(rb_sems[n % 2], 16)
                sync.wait_ge(ob_sem, q + 1)
                sync.dma_start(
                    out=out_ext[:, :, q * 1024:(q + 1) * 1024],
                    in_=ob_sb.ap(),
                ).then_inc(out_sem, 16)
            sync.wait_ge(out_sem, 64)

        @block.tensor
        def _(tensor):
            tensor.wait_ge(in_sem, 80)
            for h in range(2):
                if h == 1:
                    tensor.wait_ge(pm_sem, 1)
                for j in range(16):
                    for ch in range(2):
                        mm = tensor.matmul(
                            ps_sh.ap()[:, j, 0:72],
                            xo_sb.ap()[:, ch, (16 * h + j) * 128:(16 * h + j + 1) * 128],
                            wo_sb.ap()[:, ch, :],
                            start=(ch == 0),
                            stop=(ch == 1),
                        )
                        if j == 15 and ch == 1:
                            mm.then_inc(off_sem, 1)
            for t in range(T):
                for i in range(NBLK):
                    g = t * NBLK + i
                    if i == 0:
                        tensor.wait_ge(lerp_sem, 384 * (t + 1))
                    if g >= 2:
                        tensor.wait_ge(ev_sem, g - 1)
                    tensor.transpose(
                        ps_tr[g % 2].ap()[:, 0:128],
                        samp[t % 2].ap()[:, i, :],
                        id_sb.ap(),
                    ).then_inc(tr_sem, 1)
            for q in range(4):
                for t in range(T):
                    n = q * T + t
                    tensor.wait_ge(rb_sems[n % 2], 16 * (n // 2 + 1))
                    if t == 0 and q >= 1:
                        tensor.wait_ge(ob_sem, q)
                    for oc in range(2):
                        for hh in range(2):
                            mm = tensor.matmul(
                                ps_out[:, oc * 2 + hh, :],
                                wd_sb.ap()[:, t, oc * 128:(oc + 1) * 128],
                                rb_sb[n % 2].ap()[:, hh * 512:(hh + 1) * 512],
                                start=(t == 0),
                                stop=(t == 17),
                            )
                            if oc == 1 and hh == 1:
                                mm.then_inc(mm_sem, 1)

        @block.vector
        def _(vector):
            vector.wait_ge(in_sem, 80)
            for h in range(2):
                vector.wait_ge(off_sem, h + 1)
                vector.tensor_tensor(
                    out=p_ap[:, 16 * h:16 * (h + 1), :],
                    in0=ps_sh.ap()[:, :, 0:72],
                    in1=gb_sb.ap()[:, 16 * h:16 * (h + 1), :],
                    op=mybir.AluOpType.add,
                ).then_inc(pm_sem, 1)
            vector.drain()
            vector.tensor_scalar(
                out=fi_ap, in0=p_ap, scalar1=-0.5, scalar2=None,
                op0=mybir.AluOpType.add)
            vector.drain()
            vector.tensor_copy(out=ff_ap, in_=fi_ap)
            vector.drain()
            vector.tensor_tensor(
                out=wf_sb.ap(), in0=p_ap, in1=ff_ap,
                op=mybir.AluOpType.subtract)
            vector.drain()
            vector.tensor_scalar(
                out=ff_ap[:, :, 0:36], in0=ff_ap[:, :, 0:36],
                scalar1=-60.0, scalar2=0.0,
                op0=mybir.AluOpType.add, op1=mybir.AluOpType.max)
            vector.drain()
            vector.tensor_scalar(
                out=ff_ap[:, :, 0:36], in0=ff_ap[:, :, 0:36],
                scalar1=70.0, scalar2=None, op0=mybir.AluOpType.min)
            vector.tensor_scalar(
                out=ff_ap[:, :, 36:72], in0=ff_ap[:, :, 36:72],
                scalar1=-64.0, scalar2=0.0,
                op0=mybir.AluOpType.add, op1=mybir.AluOpType.max)
            vector.drain()
            vector.tensor_scalar(
                out=ff_ap[:, :, 36:72], in0=ff_ap[:, :, 36:72],
                scalar1=71.0, scalar2=None, op0=mybir.AluOpType.min)
            vector.drain()
            vector.scalar_tensor_tensor(
                out=li_sb.ap(), in0=ff_ap[:, :, 0:36], scalar=73.0,
                in1=ff_ap[:, :, 36:72],
                op0=mybir.AluOpType.mult, op1=mybir.AluOpType.add)
            vector.drain()
            vector.tensor_copy(out=li16_sb.ap(), in_=li_sb.ap()).then_inc(idx_sem, 1)
            # hop 2 of idx wrap: idxw[p, t, 8i+u] = idxI[p, u, t, i]
            vector.wait_ge(wrap_sem, 16 * 64)
            vector.drain()
            src = bass.AP(
                tensor=xo_i, offset=0,
                ap=[[16384, 128], [1, 36], [36, 32], [36 * 32, 8]])
            dst = bass.AP(
                tensor=arena2_i, offset=0,
                ap=[[11520, 128], [256, 36], [8, 32], [1, 8]])
            vector.tensor_copy(out=dst, in_=src).then_inc(idx_sem, 1)

            vs = [0]

            def vsync(last):
                last.then_inc(vs_sem, 1)
                vs[0] += 1
                vector.wait_ge(vs_sem, vs[0])

            for t in range(T):
                k2, gp = t // 2, t % 2
                for n_ in (2 * t, 2 * t + 1):
                    vector.wait_ge(g_sems[n_ % 4], 16 * (n_ // 4 + 1))
                if t >= 2:
                    vector.wait_ge(tr_sem, NBLK * (t - 1))
                sp_ = samp[t % 2].ap()
                gaps = [gbuf[(t * 2 + idg) % NGB].ap() for idg in range(2)]
                taps = [(gp * 2 + idg) * 9 + k2 for idg in range(2)]

                def wxb(idg, width):
                    return bass.AP(tensor=wf_sb, offset=36 + taps[idg],
                                   ap=[[2304, 128], [72, 32], [0, width]])

                def wyb(idg, width):
                    return bass.AP(tensor=wf_sb, offset=taps[idg],
                                   ap=[[2304, 128], [72, 32], [0, width]])

                # s0: dpair = [e01|e11] - [e00|e10]  (into slots 128:256)
                last = None
                for idg in range(2):
                    g_ = gaps[idg]
                    last = vector.tensor_tensor(
                        out=g_[:, :, 128:256], in0=g_[:, :, 128:256],
                        in1=g_[:, :, 0:128], op=mybir.AluOpType.subtract)
                vsync(last)
                # s1a: dpair *= wx
                for idg in range(2):
                    g_ = gaps[idg]
                    last = vector.tensor_tensor(
                        out=g_[:, :, 128:256], in0=g_[:, :, 128:256],
                        in1=wxb(idg, 128), op=mybir.AluOpType.mult)
                vsync(last)
                # s1b: upair = dpair + [e00|e10]
                for idg in range(2):
                    g_ = gaps[idg]
                    last = vector.tensor_tensor(
                        out=g_[:, :, 128:256], in0=g_[:, :, 128:256],
                        in1=g_[:, :, 0:128], op=mybir.AluOpType.add)
                vsync(last)
                # s2: dv = u1 - u0 (slots 192:256 -= 128:192)
                for idg in range(2):
                    g_ = gaps[idg]
                    last = vector.tensor_tensor(
                        out=g_[:, :, 192:256], in0=g_[:, :, 192:256],
                        in1=g_[:, :, 128:192], op=mybir.AluOpType.subtract)
                vsync(last)
                # s3a: dv *= wy
                for idg in range(2):
                    g_ = gaps[idg]
                    last = vector.tensor_tensor(
                        out=g_[:, :, 192:256], in0=g_[:, :, 192:256],
                        in1=wyb(idg, 64), op=mybir.AluOpType.mult)
                vsync(last)
                # s3b: samp = dv + u0
                for idg in range(2):
                    g_ = gaps[idg]
                    last = vector.tensor_tensor(
                        out=sp_[:, :, idg * 64:(idg + 1) * 64],
                        in0=g_[:, :, 192:256], in1=g_[:, :, 128:192],
                        op=mybir.AluOpType.add)
                last.then_inc(lerp_sem, 384)

        @block.scalar
        def _(scalar):
            for t in range(T):
                for i in range(NBLK):
                    g = t * NBLK + i
                    scalar.wait_ge(tr_sem, g + 1)
                    if t >= 2 and i == 0:
                        scalar.wait_ge(stw_sem, 16 * (t - 1))
                    scalar.activation(
                        out=st_aps[t % 2][:, i * 128:(i + 1) * 128],
                        in_=ps_tr[g % 2].ap()[:, 0:128],
                        func=mybir.ActivationFunctionType.Copy,
                    ).then_inc(ev_sem, 1)
            for q in range(4):
                scalar.wait_ge(mm_sem, T * (q + 1))
                if q >= 1:
                    scalar.wait_ge(out_sem, 16 * q)
                scalar.activation(
                    out=ob_sb.ap().rearrange("p a b -> p (a b)"),
                    in_=ps_out.rearrange("p r n -> p (r n)"),
                    func=mybir.ActivationFunctionType.Relu,
                ).then_inc(ob_sem, 1)

        @block.gpsimd
        def _(gpsimd):
            gpsimd.load_library(mlp)
            gpsimd.wait_ge(idx_sem, 2)
            for t in range(T):
                k2, gp = t // 2, t % 2
                for idg in range(2):
                    dg = gp * 2 + idg
                    n = t * 2 + idg
                    if n >= NGB:
                        tm = (n - NGB) // 2
                        gpsimd.wait_ge(lerp_sem, 384 * (tm + 1))
                    tap = dg * 9 + k2
                    gpsimd.dma_gather(
                        out_ap=gbuf[n % NGB].ap(),
                        in_ap=ptab[dg],
                        idxs_ap=idxw_ap[:, tap, :],
                        num_idxs=NPIX,
                        num_idxs_reg=NPIX,
                        elem_size=256,
                        queue_num=n % 4,
                        single_packet=False,
                    ).then_inc(g_sems[n % 4], 16)


    nc.compile()
    return nc


# ---------------------------------------------------------------------------
# host side
# ---------------------------------------------------------------------------

def _host_prep(x0, x1, x2, w_off, b_off, w_def):
    x = np.concatenate([np.asarray(x0), np.asarray(x1), np.asarray(x2)],
                       axis=1).astype(np.float32)
    w_off = np.asarray(w_off, dtype=np.float32)
    b_off = np.asarray(b_off, dtype=np.float32)
    w_def = np.asarray(w_def, dtype=np.float32)

    perm = np.empty(72, dtype=np.int64)
    for dg in range(DG):
        for k2 in range(K2):
            perm[dg * 9 + k2] = (dg * 9 + k2) * 2
            perm[36 + dg * 9 + k2] = (dg * 9 + k2) * 2 + 1
    w_off_perm = w_off[perm]
    b_off_perm = b_off[perm]

    ky = np.repeat(np.arange(3), 3).astype(np.float32)
    kx = np.tile(np.arange(3), 3).astype(np.float32)

    w_off_p = np.empty((128, 2, 72), dtype=np.float32)
    for ch in range(2):
        w_off_p[:, ch, :] = w_off_perm[:, ch * 128:(ch + 1) * 128].T

    pix = np.arange(32)[None, :] * 128 + np.arange(128)[:, None]
    hh = (pix // WB).astype(np.float32)
    wl = (pix % WB).astype(np.float32)
    gridbias = np.empty((128, 32, 72), dtype=np.float32)
    for dg in range(DG):
        for k2 in range(K2):
            o = dg * 9 + k2
            gridbias[:, :, o] = hh + (ky[k2] + 63.0 + b_off_perm[o])
            gridbias[:, :, 36 + o] = wl + (kx[k2] + 67.0 + b_off_perm[36 + o])

    w_def_p = np.empty((128, 18, 256), dtype=np.float32)
    for k2 in range(K2):
        for gp in range(2):
            w_def_p[:, k2 * 2 + gp, :] = w_def[:, gp * 128:(gp + 1) * 128, k2].T
    w_def_p = w_def_p.astype(bf16)

    ident = np.eye(128, dtype=np.float32).astype(bf16)

    in_maps = []
    for core in range(8):
        b, wb = core // 4, core % 4
        ws = wb * WB
        strip = np.zeros((C, HS, TW + 1), dtype=np.float32)
        lo, hi = ws - PAD, ws - PAD + TW + 1
        slo, shi = max(0, lo), min(W, hi)
        strip[:, PAD:PAD + H, slo - lo:shi - lo] = x[b, :, :, slo:shi]
        strip_bf = strip.astype(bf16)

        ptab = np.zeros((DG, HS, TW, 2, 2, CG), dtype=bf16)
        for dg in range(DG):
            sc = np.moveaxis(strip_bf[dg * CG:(dg + 1) * CG], 0, -1)
            scp = np.zeros((HS + 1, TW + 1, CG), dtype=bf16)
            scp[:HS] = sc
            ptab[dg, :, :, 0, 0] = scp[:HS, :TW]        # e00
            ptab[dg, :, :, 0, 1] = scp[1:HS + 1, :TW]    # e10
            ptab[dg, :, :, 1, 0] = scp[:HS, 1:TW + 1]    # e01
            ptab[dg, :, :, 1, 1] = scp[1:HS + 1, 1:TW + 1]  # e11
        ptab = ptab.reshape(DG, NROWS, 256)

        xo = x[b, :, :, ws:ws + WB].reshape(C, NPIX)
        x_off = np.empty((128, 2, NPIX), dtype=np.float32)
        for ch in range(2):
            x_off[:, ch, :] = xo[ch * 128:(ch + 1) * 128]

        in_maps.append({
            "x_off": np.ascontiguousarray(x_off),
            "ptab": np.ascontiguousarray(ptab),
            "w_off_p": np.ascontiguousarray(w_off_p),
            "gridbias": np.ascontiguousarray(gridbias),
            "w_def_p": np.ascontiguousarray(w_def_p),
            "ident": ident,
        })
    return in_maps


def _assemble(outs):
    full = np.empty((B, C_OUT, H, W), dtype=np.float32)
    for core in range(8):
        b, wb = core // 4, core % 4
        o = np.asarray(outs[core])
        for oc in range(2):
            full[b, oc * 128:(oc + 1) * 128, :, wb * WB:(wb + 1) * WB] = \
                o[:, oc, :].reshape(128, H, WB)
    return full


def kernel(**inputs):
    sys.path.insert(0, "/root/problem")
    try:
        import axon_shim  # noqa: F401  (enables NTFF profiling when present)
    except Exception:
        pass
    from concourse.bass_utils import run_bass_kernel_spmd

    in_maps = _host_prep(**inputs)
    if "nc" not in _GRAPH_CACHE:
        _GRAPH_CACHE["nc"] = build_graph()
    nc = _GRAPH_CACHE["nc"]
    res = run_bass_kernel_spmd(nc, in_maps, list(range(8)))
    return _assemble([r["out"] for r in res.results])
